# revision 14
# baseline (speedup 1.0000x reference)
"""DenseGCN (multi-edge-type) Trainium2 kernel.

Data-parallel over batch across 8 NeuronCores (8 graphs per core).

Math (per graph):
  adj_sl = adj with diagonal set to 1 (self loops), per edge type f
  deg[i,f] = clip(sum_j adj_sl[i,j,f], 1)^-0.5
  layer(h) = silu((sum_f D_f adj_sl_f D_f) @ (h W) + b) * mask
  Collapse edge types:  A2[i,j] = sum_f deg[i,f]*adj[i,j,f]*deg[j,f]
  and the self-loop diagonal correction becomes a per-row scalar:
  Cs[i] = sum_f deg[i,f]^2 * (1 - adj[i,i,f]),  so
  layer(h) = silu(A2raw @ hW + Cs*hW + b) * mask.

Per-graph pipeline (two-stage software pipeline across graphs):
  stage_load:    contiguous adj DMA -> deg row-sums on ScalarE (accum_out)
                 -> Newton rsqrt on DVE (no ACT table switch) -> in-place
                 deg_i row scale on GPSIMD; x transpose + x@W0 on PE.
  stage_compute: PE 128x128 block transposes of the scaled adj -> fused
                 deg_j-scale+edge-sum on DVE producing A2T [j part, i free]
                 -> per-layer matmul (h chunks stationary, A2T moving) ->
                 PE transpose back to natural layout with the bias folded
                 in as a rank-1 matmul accumulate -> sigmoid epilogue ->
                 masked mean pool via matmul -> tiny MLP head.
"""

import os

import numpy as np

import concourse.bass as bass
from concourse import bacc, masks, mybir, tile
from concourse.bass_utils import run_bass_kernel_spmd

B, N, F = 64, 512, 4
IN, H, OUT = 64, 128, 16
NCORES = 8
BPC = B // NCORES  # graphs per core
P = 128
NCH = N // P  # 4 chunks of 128 nodes

dt = mybir.dt
AF = mybir.ActivationFunctionType
ALU = mybir.AluOpType
AXL = mybir.AxisListType

F32R_MM = os.environ.get("GCN_F32R_MM", "1") == "1"


def build_nc(n_batches=BPC):
    nc = bacc.Bacc(
        "TRN2", target_bir_lowering=False, debug=False, enable_asserts=False
    )

    adj_d = nc.dram_tensor(
        "adj", [n_batches, N, N, F], dt.float32, kind="ExternalInput"
    )
    x_d = nc.dram_tensor("x", [n_batches, N, IN], dt.float32, kind="ExternalInput")
    mask_d = nc.dram_tensor("mask", [n_batches, N], dt.int32, kind="ExternalInput")
    W0_d = nc.dram_tensor("W0", [IN, H], dt.float32, kind="ExternalInput")
    b0_d = nc.dram_tensor("b0", [H], dt.float32, kind="ExternalInput")
    W1_d = nc.dram_tensor("W1", [H, H], dt.float32, kind="ExternalInput")
    b1_d = nc.dram_tensor("b1", [H], dt.float32, kind="ExternalInput")
    Wl1_d = nc.dram_tensor("Wl1", [H, H], dt.float32, kind="ExternalInput")
    bl1_d = nc.dram_tensor("bl1", [H], dt.float32, kind="ExternalInput")
    Wl2_d = nc.dram_tensor("Wl2", [H, OUT], dt.float32, kind="ExternalInput")
    bl2_d = nc.dram_tensor("bl2", [OUT], dt.float32, kind="ExternalInput")
    out_d = nc.dram_tensor("out", [n_batches, OUT], dt.float32, kind="ExternalOutput")

    a2dt = dt.float32r if F32R_MM else dt.float32

    with tile.TileContext(nc) as tc:
        with (
            tc.tile_pool(name="const", bufs=1) as constp,
            tc.tile_pool(name="adjp", bufs=3 * NCH) as adjp,
            tc.tile_pool(name="a2p", bufs=3 * NCH) as a2p,
            tc.tile_pool(name="hp", bufs=3) as hp,
            tc.tile_pool(name="smallp", bufs=3) as smallp,
            tc.tile_pool(name="medp", bufs=2) as medp,
            tc.tile_pool(name="psA", bufs=2, space="PSUM") as psA,
            tc.tile_pool(name="psB", bufs=3, space="PSUM") as psB,
            tc.tile_pool(name="psC", bufs=2, space="PSUM") as psC,
            tc.tile_pool(name="psD", bufs=1, space="PSUM") as psD,
        ):
            ident = constp.tile([P, P], dt.float32)
            masks.make_identity(nc, ident[:])
            W0s = constp.tile([IN, H], dt.float32)
            nc.sync.dma_start(W0s[:], W0_d.ap())
            W1s = constp.tile([H, H], dt.float32)
            nc.sync.dma_start(W1s[:], W1_d.ap())
            Wl1s = constp.tile([H, H], dt.float32)
            nc.sync.dma_start(Wl1s[:], Wl1_d.ap())
            Wl2s = constp.tile([H, OUT], dt.float32)
            nc.sync.dma_start(Wl2s[:], Wl2_d.ap())
            # conv biases as single-partition rows (folded into the PSUM
            # back-transpose group as a rank-1 matmul accumulate)
            b0r = constp.tile([1, H], dt.float32)
            nc.sync.dma_start(b0r[:], b0_d.ap().rearrange("(o h) -> o h", o=1))
            b1r = constp.tile([1, H], dt.float32)
            nc.sync.dma_start(b1r[:], b1_d.ap().rearrange("(o h) -> o h", o=1))
            ones1 = constp.tile([1, P], dt.float32)
            nc.vector.memset(ones1[:], 1.0)
            bl1c = constp.tile([H, 1], dt.float32)
            nc.sync.dma_start(bl1c[:], bl1_d.ap().rearrange("(p o) -> p o", o=1))
            bl2c = constp.tile([OUT, 1], dt.float32)
            nc.sync.dma_start(bl2c[:], bl2_d.ap().rearrange("(p o) -> p o", o=1))
            outS = constp.tile([OUT, n_batches], dt.float32)

            def stage_load(b):
                st = {}
                adjN = []
                for ci in range(NCH):
                    t = adjp.tile([P, N, F], dt.float32, tag="adjN")
                    nc.sync.dma_start(t[:], adj_d.ap()[b, ci * P : (ci + 1) * P])
                    adjN.append(t)
                st["adjN"] = adjN
                diagN = smallp.tile([P, NCH, F], dt.float32, tag="diag")
                nc.sync.dma_start(
                    diagN[:],
                    bass.AP(
                        tensor=adj_d,
                        offset=b * N * N * F,
                        ap=[[(N * F + F), P], [(N * F + F) * P, NCH], [1, F]],
                    ),
                )
                xb = smallp.tile([P, NCH, IN], dt.float32, tag="xb")
                nc.sync.dma_start(
                    xb[:], x_d.ap()[b].rearrange("(c p) d -> p c d", p=P)
                )
                mi = smallp.tile([P, NCH], dt.int32, tag="mi")
                nc.sync.dma_start(
                    mi[:], mask_d.ap()[b].rearrange("(c p) -> p c", p=P)
                )
                maskb = smallp.tile([P, NCH], dt.float32, tag="maskb")
                nc.vector.tensor_copy(maskb[:], mi[:])
                st["maskb"] = maskb

                # degrees: row sums on ScalarE via activation accumulate
                degsum = smallp.tile([P, NCH, F], dt.float32, tag="degsum")
                junk = medp.tile([P, N], dt.float32, tag="junk")
                for ci in range(NCH):
                    for f in range(F):
                        nc.scalar.activation(
                            junk[:],
                            adjN[ci][:, :, f],
                            AF.Copy,
                            accum_out=degsum[:, ci, f : f + 1],
                        )
                # deg = (max(degsum + 1 - diag, 1))^-0.5
                dtmp = smallp.tile([P, NCH, F], dt.float32, tag="dtmp")
                nc.vector.tensor_tensor(dtmp[:], degsum[:], diagN[:], ALU.subtract)
                nc.vector.tensor_scalar(dtmp[:], dtmp[:], 1.0, 1.0, ALU.add, ALU.max)
                # deg = dtmp^-0.5 via fast-inverse-sqrt + 3 Newton steps
                # (keeps ScalarE on a single activation table all kernel)
                ibits = smallp.tile([P, NCH, F], dt.int32, tag="ibits")
                nc.vector.tensor_scalar(
                    ibits[:],
                    dtmp[:].bitcast(dt.int32),
                    1,
                    None,
                    op0=ALU.logical_shift_right,
                )
                nc.vector.tensor_scalar(
                    ibits[:], ibits[:], -1, 0x5F3759DF, op0=ALU.mult, op1=ALU.add
                )
                deg = smallp.tile([P, NCH, F], dt.float32, tag="deg")
                dnt = smallp.tile([P, NCH, F], dt.float32, tag="dnt")
                cur = ibits[:].bitcast(dt.float32)
                for _ in range(3):
                    nc.vector.tensor_tensor(dnt[:], cur, cur, ALU.mult)
                    nc.vector.tensor_tensor(dnt[:], dnt[:], dtmp[:], ALU.mult)
                    nc.vector.tensor_scalar(
                        dnt[:], dnt[:], -0.5, 1.5, op0=ALU.mult, op1=ALU.add
                    )
                    nc.vector.tensor_tensor(deg[:], cur, dnt[:], ALU.mult)
                    cur = deg[:]
                st["deg"] = deg
                # row scale adj by deg_i (GPSIMD, in place)
                for ci in range(NCH):
                    for f in range(F):
                        nc.gpsimd.tensor_scalar_mul(
                            adjN[ci][:, :, f],
                            adjN[ci][:, :, f],
                            deg[:, ci, f : f + 1],
                        )
                # Cs = sum_f deg^2 * (1 - diag)
                om = smallp.tile([P, NCH, F], dt.float32, tag="om")
                nc.vector.tensor_scalar(
                    om[:], diagN[:], -1.0, 1.0, ALU.mult, ALU.add
                )
                csf = smallp.tile([P, NCH, F], dt.float32, tag="csf")
                nc.vector.tensor_tensor(csf[:], deg[:], deg[:], ALU.mult)
                nc.vector.tensor_tensor(csf[:], csf[:], om[:], ALU.mult)
                Cs = smallp.tile([P, NCH], dt.float32, tag="Cs")
                nc.vector.tensor_reduce(Cs[:], csf[:], axis=AXL.X, op=ALU.add)
                st["Cs"] = Cs
                maskdiv = smallp.tile([P, NCH], dt.float32, tag="md")
                nc.vector.tensor_scalar_mul(maskdiv[:], maskb[:], 1.0 / N)
                st["maskdiv"] = maskdiv

                # h0 = x @ W0 (natural [i, c] layout)
                psX = psC.tile([IN, N], dt.float32, tag="px")
                for ci in range(NCH):
                    nc.tensor.transpose(
                        psX[:, ci * P : (ci + 1) * P], xb[:, ci, :], ident[:]
                    )
                xTs = medp.tile([IN, N], dt.float32, tag="xTs")
                nc.scalar.copy(xTs[:], psX[:])
                psH0 = psC.tile([P, NCH, H], dt.float32, tag="px")
                for ci in range(NCH):
                    nc.tensor.matmul(
                        psH0[:, ci, :],
                        xTs[:, ci * P : (ci + 1) * P],
                        W0s[:],
                        start=True,
                        stop=True,
                    )
                h0 = hp.tile([P, NCH, H], a2dt, tag="h0")
                nc.scalar.copy(h0[:], psH0[:])
                st["h0"] = h0
                return st

            def stage_compute(b, st):
                adjN = st["adjN"]
                deg = st["deg"]
                Cs = st["Cs"]
                maskb = st["maskb"]
                maskdiv = st["maskdiv"]

                # transpose + assemble A2T [j, i]
                A2T = []
                for cj in range(NCH):
                    acc = a2p.tile([P, N], a2dt, tag="A2T")
                    for f in range(F):
                        BT = psA.tile([P, N], dt.float32, tag="BT")
                        for ci in range(NCH):
                            nc.tensor.transpose(
                                BT[:, ci * P : (ci + 1) * P],
                                adjN[ci][:, cj * P : (cj + 1) * P, f],
                                ident[:],
                            )
                        if f == 0:
                            nc.vector.tensor_scalar_mul(
                                acc[:], BT[:], deg[:, cj, 0:1]
                            )
                        else:
                            nc.vector.scalar_tensor_tensor(
                                acc[:],
                                BT[:],
                                deg[:, cj, f : f + 1],
                                acc[:],
                                op0=ALU.mult,
                                op1=ALU.add,
                            )
                    A2T.append(acc)

                # two GCN layers
                hin = st["h0"]  # already x @ W0
                for l in range(2):
                    bR = b0r if l == 0 else b1r
                    if l == 0:
                        hw = hin
                    else:
                        # hw = h1 @ W1: transpose h1, then W1 matmuls
                        psT = psB.tile([H, N], dt.float32, tag="mm")
                        for ci in range(NCH):
                            nc.tensor.transpose(
                                psT[:, ci * P : (ci + 1) * P],
                                hin[:, ci, :],
                                ident[:],
                            )
                        hTs = medp.tile([H, N], dt.float32, tag="hTs")
                        nc.scalar.copy(hTs[:], psT[:])
                        psW = psB.tile([P, NCH, H], dt.float32, tag="mm")
                        for ci in range(NCH):
                            nc.tensor.matmul(
                                psW[:, ci, :],
                                hTs[:, ci * P : (ci + 1) * P],
                                W1s[:],
                                start=True,
                                stop=True,
                            )
                        hw = hp.tile([P, NCH, H], a2dt, tag="hw")
                        nc.scalar.copy(hw[:], psW[:])
                    psL = psB.tile([H, N], dt.float32, tag="mm")
                    for cj in range(NCH):
                        nc.tensor.matmul(
                            psL[:],
                            hw[:, cj, :],
                            A2T[cj][:],
                            start=(cj == 0),
                            stop=(cj == NCH - 1),
                        )
                    MTs = medp.tile([H, N], dt.float32, tag="MTs")
                    nc.scalar.copy(MTs[:], psL[:])
                    psN = psB.tile([P, NCH, H], dt.float32, tag="mm")
                    for ci in range(NCH):
                        nc.tensor.matmul(
                            psN[:, ci, :],
                            MTs[:, ci * P : (ci + 1) * P],
                            ident[:],
                            is_transpose=True,
                            start=True,
                            stop=False,
                        )
                        nc.tensor.matmul(
                            psN[:, ci, :],
                            ones1[:],
                            bR[:],
                            start=False,
                            stop=True,
                        )
                    tmp = hp.tile([P, NCH, H], dt.float32, tag="tmp")
                    for ci in range(NCH):
                        nc.vector.scalar_tensor_tensor(
                            tmp[:, ci, :],
                            hw[:, ci, :],
                            Cs[:, ci : ci + 1],
                            psN[:, ci, :],
                            op0=ALU.mult,
                            op1=ALU.add,
                        )
                    # silu(x) = x * sigmoid(x); layer-1 mask folded into the
                    # multiply, layer-2 mask folded into the pooling vector
                    sg = hp.tile([P, NCH, H], dt.float32, tag="sg")
                    nc.scalar.activation(sg[:], tmp[:], AF.Sigmoid)
                    hout = hp.tile([P, NCH, H], dt.float32, tag=f"h{l + 1}")
                    if l == 0:
                        for ci in range(NCH):
                            nc.vector.scalar_tensor_tensor(
                                hout[:, ci, :],
                                tmp[:, ci, :],
                                maskb[:, ci : ci + 1],
                                sg[:, ci, :],
                                op0=ALU.mult,
                                op1=ALU.mult,
                            )
                    else:
                        nc.vector.tensor_tensor(hout[:], tmp[:], sg[:], ALU.mult)
                    hin = hout

                # masked mean pool + MLP head
                psG = psD.tile([1, H], dt.float32, tag="head")
                for ci in range(NCH):
                    nc.tensor.matmul(
                        psG[:],
                        maskdiv[:, ci : ci + 1],
                        hin[:, ci, :],
                        start=(ci == 0),
                        stop=(ci == NCH - 1),
                    )
                gs = smallp.tile([1, H], dt.float32, tag="gs")
                nc.scalar.copy(gs[:], psG[:])
                psGT = psD.tile([H, 1], dt.float32, tag="head")
                nc.tensor.transpose(psGT[:], gs[:], ident[0:1, 0:1])
                gT = smallp.tile([H, 1], dt.float32, tag="gT")
                nc.scalar.copy(gT[:], psGT[:])
                psH1 = psD.tile([H, 1], dt.float32, tag="head")
                nc.tensor.matmul(psH1[:], Wl1s[:], gT[:], start=True, stop=True)
                g1pre = smallp.tile([H, 1], dt.float32, tag="g1pre")
                nc.scalar.activation(
                    g1pre[:], psH1[:], AF.Identity, bias=bl1c[:, 0:1]
                )
                g1sg = smallp.tile([H, 1], dt.float32, tag="g1sg")
                nc.scalar.activation(
                    g1sg[:], psH1[:], AF.Sigmoid, bias=bl1c[:, 0:1]
                )
                g1 = smallp.tile([H, 1], dt.float32, tag="g1")
                nc.vector.tensor_tensor(g1[:], g1pre[:], g1sg[:], ALU.mult)
                psO = psD.tile([OUT, 1], dt.float32, tag="head")
                nc.tensor.matmul(psO[:], Wl2s[:], g1[:], start=True, stop=True)
                nc.scalar.activation(
                    outS[:, b : b + 1], psO[:], AF.Identity, bias=bl2c[:, 0:1]
                )

            prev = None
            for b in range(n_batches):
                st = stage_load(b)
                if prev is not None:
                    stage_compute(b - 1, prev)
                prev = st
            stage_compute(n_batches - 1, prev)

            nc.sync.dma_start(out_d.ap().rearrange("b c -> c b"), outS[:])

    nc.compile()
    return nc


_NC_CACHE = {}


def _get_nc(n_batches=BPC):
    if n_batches not in _NC_CACHE:
        _NC_CACHE[n_batches] = build_nc(n_batches)
    return _NC_CACHE[n_batches]


def make_in_maps(x, adj, mask, W0, b0, W1, b1, Wl1, bl1, Wl2, bl2):
    ws = dict(
        W0=np.ascontiguousarray(W0, np.float32),
        b0=np.ascontiguousarray(b0, np.float32),
        W1=np.ascontiguousarray(W1, np.float32),
        b1=np.ascontiguousarray(b1, np.float32),
        Wl1=np.ascontiguousarray(Wl1, np.float32),
        bl1=np.ascontiguousarray(bl1, np.float32),
        Wl2=np.ascontiguousarray(Wl2, np.float32),
        bl2=np.ascontiguousarray(bl2, np.float32),
    )
    in_maps = []
    for c in range(NCORES):
        sl = slice(c * BPC, (c + 1) * BPC)
        m = dict(
            adj=np.ascontiguousarray(adj[sl], np.float32),
            x=np.ascontiguousarray(x[sl], np.float32),
            mask=np.ascontiguousarray(mask[sl], np.int32),
        )
        m.update(ws)
        in_maps.append(m)
    return in_maps


def kernel(x, adj, mask, W0, b0, W1, b1, Wl1, bl1, Wl2, bl2, **kw):
    nc = _get_nc()
    in_maps = make_in_maps(x, adj, mask, W0, b0, W1, b1, Wl1, bl1, Wl2, bl2)
    res = run_bass_kernel_spmd(nc, in_maps, core_ids=list(range(NCORES)))
    out = np.concatenate([res.results[c]["out"] for c in range(NCORES)], axis=0)
    return out.astype(np.float32)


# revision 15
# speedup vs baseline: 3.2571x; 3.2571x over previous
"""DenseGCN (multi-edge-type) Trainium2 kernel.

Data-parallel over batch across 8 NeuronCores (8 graphs per core).

Math (per graph):
  adj_sl = adj with diagonal set to 1 (self loops), per edge type f
  deg[i,f] = clip(sum_j adj_sl[i,j,f], 1)^-0.5
  layer(h) = silu((sum_f D_f adj_sl_f D_f) @ (h W) + b) * mask
  Collapse edge types:  A2[i,j] = sum_f deg[i,f]*adj[i,j,f]*deg[j,f]
  and the self-loop diagonal correction becomes a per-row scalar:
  Cs[i] = sum_f deg[i,f]^2 * (1 - adj[i,i,f]),  so
  layer(h) = silu(A2raw @ hW + Cs*hW + b) * mask.

Per-graph pipeline (two-stage software pipeline across graphs):
  stage_load:    contiguous adj DMA -> deg row-sums on ScalarE (accum_out)
                 -> Newton rsqrt on DVE (no ACT table switch) -> in-place
                 deg_i row scale on GPSIMD; x transpose + x@W0 on PE.
  stage_compute: PE 128x128 block transposes of the scaled adj -> fused
                 deg_j-scale+edge-sum on DVE producing A2T [j part, i free]
                 -> per-layer matmul (h chunks stationary, A2T moving) ->
                 PE transpose back to natural layout with the bias folded
                 in as a rank-1 matmul accumulate -> sigmoid epilogue ->
                 masked mean pool via matmul -> tiny MLP head.
"""

import os

import numpy as np

import concourse.bass as bass
from concourse import bacc, masks, mybir, tile
from concourse.bass_utils import run_bass_kernel_spmd

B, N, F = 64, 512, 4
IN, H, OUT = 64, 128, 16
NCORES = 8
BPC = B // NCORES  # graphs per core
P = 128
NCH = N // P  # 4 chunks of 128 nodes

dt = mybir.dt
AF = mybir.ActivationFunctionType
ALU = mybir.AluOpType
AXL = mybir.AxisListType

F32R_MM = os.environ.get("GCN_F32R_MM", "1") == "1"


def build_nc(n_batches=BPC):
    nc = bacc.Bacc(
        "TRN2", target_bir_lowering=False, debug=False, enable_asserts=False
    )

    adj_d = nc.dram_tensor(
        "adj", [n_batches, N, N, F], dt.float32, kind="ExternalInput"
    )
    x_d = nc.dram_tensor("x", [n_batches, N, IN], dt.float32, kind="ExternalInput")
    mask_d = nc.dram_tensor("mask", [n_batches, N], dt.int32, kind="ExternalInput")
    W0_d = nc.dram_tensor("W0", [IN, H], dt.float32, kind="ExternalInput")
    b0_d = nc.dram_tensor("b0", [H], dt.float32, kind="ExternalInput")
    W1_d = nc.dram_tensor("W1", [H, H], dt.float32, kind="ExternalInput")
    b1_d = nc.dram_tensor("b1", [H], dt.float32, kind="ExternalInput")
    Wl1_d = nc.dram_tensor("Wl1", [H, H], dt.float32, kind="ExternalInput")
    bl1_d = nc.dram_tensor("bl1", [H], dt.float32, kind="ExternalInput")
    Wl2_d = nc.dram_tensor("Wl2", [H, OUT], dt.float32, kind="ExternalInput")
    bl2_d = nc.dram_tensor("bl2", [OUT], dt.float32, kind="ExternalInput")
    out_d = nc.dram_tensor("out", [n_batches, OUT], dt.float32, kind="ExternalOutput")

    a2dt = dt.float32r if F32R_MM else dt.float32

    with tile.TileContext(nc) as tc:
        with (
            tc.tile_pool(name="const", bufs=1) as constp,
            tc.tile_pool(name="adjp", bufs=3 * NCH) as adjp,
            tc.tile_pool(name="a2p", bufs=3 * NCH) as a2p,
            tc.tile_pool(name="hp", bufs=3) as hp,
            tc.tile_pool(name="smallp", bufs=3) as smallp,
            tc.tile_pool(name="medp", bufs=2) as medp,
            tc.tile_pool(name="psA", bufs=2, space="PSUM") as psA,
            tc.tile_pool(name="psB", bufs=3, space="PSUM") as psB,
            tc.tile_pool(name="psC", bufs=2, space="PSUM") as psC,
            tc.tile_pool(name="psD", bufs=1, space="PSUM") as psD,
        ):
            ident = constp.tile([P, P], dt.float32)
            masks.make_identity(nc, ident[:])
            W0s = constp.tile([IN, H], dt.float32)
            nc.sync.dma_start(W0s[:], W0_d.ap())
            W1s = constp.tile([H, H], dt.float32)
            nc.sync.dma_start(W1s[:], W1_d.ap())
            Wl1s = constp.tile([H, H], dt.float32)
            nc.sync.dma_start(Wl1s[:], Wl1_d.ap())
            Wl2s = constp.tile([H, OUT], dt.float32)
            nc.sync.dma_start(Wl2s[:], Wl2_d.ap())
            # conv biases broadcast across all 128 partitions
            b0B = constp.tile([P, H], dt.float32)
            nc.gpsimd.dma_start(
                b0B[:], bass.AP(tensor=b0_d, offset=0, ap=[[0, P], [1, H]])
            )
            b1B = constp.tile([P, H], dt.float32)
            nc.gpsimd.dma_start(
                b1B[:], bass.AP(tensor=b1_d, offset=0, ap=[[0, P], [1, H]])
            )
            bl1c = constp.tile([H, 1], dt.float32)
            nc.sync.dma_start(bl1c[:], bl1_d.ap().rearrange("(p o) -> p o", o=1))
            bl2c = constp.tile([OUT, 1], dt.float32)
            nc.sync.dma_start(bl2c[:], bl2_d.ap().rearrange("(p o) -> p o", o=1))
            outS = constp.tile([OUT, n_batches], dt.float32)

            def stage_load(b):
                st = {}
                adjN = []
                for ci in range(NCH):
                    t = adjp.tile([P, N, F], dt.float32, tag="adjN")
                    nc.sync.dma_start(t[:], adj_d.ap()[b, ci * P : (ci + 1) * P])
                    adjN.append(t)
                st["adjN"] = adjN
                diagN = smallp.tile([P, NCH, F], dt.float32, tag="diag")
                nc.sync.dma_start(
                    diagN[:],
                    bass.AP(
                        tensor=adj_d,
                        offset=b * N * N * F,
                        ap=[[(N * F + F), P], [(N * F + F) * P, NCH], [1, F]],
                    ),
                )
                xb = smallp.tile([P, NCH, IN], dt.float32, tag="xb")
                nc.sync.dma_start(
                    xb[:], x_d.ap()[b].rearrange("(c p) d -> p c d", p=P)
                )
                mi = smallp.tile([P, NCH], dt.int32, tag="mi")
                nc.sync.dma_start(
                    mi[:], mask_d.ap()[b].rearrange("(c p) -> p c", p=P)
                )
                maskb = smallp.tile([P, NCH], dt.float32, tag="maskb")
                nc.vector.tensor_copy(maskb[:], mi[:])
                st["maskb"] = maskb

                # degrees: row sums on ScalarE via activation accumulate
                degsum = smallp.tile([P, NCH, F], dt.float32, tag="degsum")
                junk = medp.tile([P, N], dt.float32, tag="junk")
                for ci in range(NCH):
                    for f in range(F):
                        nc.scalar.activation(
                            junk[:],
                            adjN[ci][:, :, f],
                            AF.Copy,
                            accum_out=degsum[:, ci, f : f + 1],
                        )
                # deg = (max(degsum + 1 - diag, 1))^-0.5
                dtmp = smallp.tile([P, NCH, F], dt.float32, tag="dtmp")
                nc.vector.tensor_tensor(dtmp[:], degsum[:], diagN[:], ALU.subtract)
                nc.vector.tensor_scalar(dtmp[:], dtmp[:], 1.0, 1.0, ALU.add, ALU.max)
                # deg = dtmp^-0.5 (ScalarE sqrt + accurate DVE reciprocal)
                dsq = smallp.tile([P, NCH, F], dt.float32, tag="dsq")
                nc.scalar.sqrt(dsq[:], dtmp[:])
                deg = smallp.tile([P, NCH, F], dt.float32, tag="deg")
                nc.vector.reciprocal(deg[:], dsq[:])
                st["deg"] = deg
                # row scale adj by deg_i (DVE, in place, contiguous read
                # with deg broadcast along j)
                for ci in range(NCH):
                    nc.vector.tensor_tensor(
                        adjN[ci][:, :, :],
                        adjN[ci][:, :, :],
                        deg[:, ci, None, :].to_broadcast([P, N, F]),
                        ALU.mult,
                    )
                # Cs = sum_f deg^2 * (1 - diag)
                om = smallp.tile([P, NCH, F], dt.float32, tag="om")
                nc.vector.tensor_scalar(
                    om[:], diagN[:], -1.0, 1.0, ALU.mult, ALU.add
                )
                csf = smallp.tile([P, NCH, F], dt.float32, tag="csf")
                nc.vector.tensor_tensor(csf[:], deg[:], deg[:], ALU.mult)
                nc.vector.tensor_tensor(csf[:], csf[:], om[:], ALU.mult)
                Cs = smallp.tile([P, NCH], dt.float32, tag="Cs")
                nc.vector.tensor_reduce(Cs[:], csf[:], axis=AXL.X, op=ALU.add)
                st["Cs"] = Cs
                maskdiv = smallp.tile([P, NCH], dt.float32, tag="md")
                nc.vector.tensor_scalar_mul(maskdiv[:], maskb[:], 1.0 / N)
                st["maskdiv"] = maskdiv

                # h0 = x @ W0 (natural [i, c] layout)
                psX = psC.tile([IN, N], dt.float32, tag="px")
                for ci in range(NCH):
                    nc.tensor.transpose(
                        psX[:, ci * P : (ci + 1) * P], xb[:, ci, :], ident[:]
                    )
                xTs = medp.tile([IN, N], dt.float32, tag="xTs")
                nc.scalar.copy(xTs[:], psX[:])
                psH0 = psC.tile([P, NCH, H], dt.float32, tag="px")
                for ci in range(NCH):
                    nc.tensor.matmul(
                        psH0[:, ci, :],
                        xTs[:, ci * P : (ci + 1) * P],
                        W0s[:],
                        start=True,
                        stop=True,
                    )
                h0 = hp.tile([P, NCH, H], a2dt, tag="h0")
                nc.scalar.copy(h0[:], psH0[:])
                st["h0"] = h0
                return st

            def stage_compute(b, st):
                adjN = st["adjN"]
                deg = st["deg"]
                Cs = st["Cs"]
                maskb = st["maskb"]
                maskdiv = st["maskdiv"]

                # transpose + assemble A2T [j, i]
                A2T = []
                for cj in range(NCH):
                    acc = a2p.tile([P, N], a2dt, tag="A2T")
                    for f in range(F):
                        BT = psA.tile([P, N], dt.float32, tag="BT")
                        for ci in range(NCH):
                            nc.tensor.transpose(
                                BT[:, ci * P : (ci + 1) * P],
                                adjN[ci][:, cj * P : (cj + 1) * P, f],
                                ident[:],
                            )
                        if f == 0:
                            nc.vector.tensor_scalar_mul(
                                acc[:], BT[:], deg[:, cj, 0:1]
                            )
                        else:
                            nc.vector.scalar_tensor_tensor(
                                acc[:],
                                BT[:],
                                deg[:, cj, f : f + 1],
                                acc[:],
                                op0=ALU.mult,
                                op1=ALU.add,
                            )
                    A2T.append(acc)

                # two GCN layers
                hin = st["h0"]  # already x @ W0
                for l in range(2):
                    bB = b0B if l == 0 else b1B
                    if l == 0:
                        hw = hin
                    else:
                        # hw = h1 @ W1: transpose h1, then W1 matmuls
                        psT = psB.tile([H, N], dt.float32, tag="mm")
                        for ci in range(NCH):
                            nc.tensor.transpose(
                                psT[:, ci * P : (ci + 1) * P],
                                hin[:, ci, :],
                                ident[:],
                            )
                        hTs = medp.tile([H, N], dt.float32, tag="hTs")
                        nc.scalar.copy(hTs[:], psT[:])
                        psW = psB.tile([P, NCH, H], dt.float32, tag="mm")
                        for ci in range(NCH):
                            nc.tensor.matmul(
                                psW[:, ci, :],
                                hTs[:, ci * P : (ci + 1) * P],
                                W1s[:],
                                start=True,
                                stop=True,
                            )
                        hw = hp.tile([P, NCH, H], a2dt, tag="hw")
                        nc.scalar.copy(hw[:], psW[:])
                    psL = psB.tile([H, N], dt.float32, tag="mm")
                    for cj in range(NCH):
                        nc.tensor.matmul(
                            psL[:],
                            hw[:, cj, :],
                            A2T[cj][:],
                            start=(cj == 0),
                            stop=(cj == NCH - 1),
                        )
                    MTs = medp.tile([H, N], dt.float32, tag="MTs")
                    nc.scalar.copy(MTs[:], psL[:])
                    psN = psB.tile([P, NCH, H], dt.float32, tag="mm")
                    for ci in range(NCH):
                        nc.tensor.transpose(
                            psN[:, ci, :], MTs[:, ci * P : (ci + 1) * P], ident[:]
                        )
                    tmp = hp.tile([P, NCH, H], dt.float32, tag="tmp")
                    for ci in range(NCH):
                        nc.vector.scalar_tensor_tensor(
                            tmp[:, ci, :],
                            hw[:, ci, :],
                            Cs[:, ci : ci + 1],
                            psN[:, ci, :],
                            op0=ALU.mult,
                            op1=ALU.add,
                        )
                    nc.vector.tensor_tensor(
                        tmp[:],
                        tmp[:],
                        bB[:, None, :].to_broadcast([P, NCH, H]),
                        ALU.add,
                    )
                    # silu(x) = x * sigmoid(x); layer-1 mask folded into the
                    # multiply, layer-2 mask folded into the pooling vector
                    sg = hp.tile([P, NCH, H], dt.float32, tag="sg")
                    nc.scalar.activation(sg[:], tmp[:], AF.Sigmoid)
                    hout = hp.tile([P, NCH, H], dt.float32, tag=f"h{l + 1}")
                    if l == 0:
                        for ci in range(NCH):
                            nc.vector.scalar_tensor_tensor(
                                hout[:, ci, :],
                                tmp[:, ci, :],
                                maskb[:, ci : ci + 1],
                                sg[:, ci, :],
                                op0=ALU.mult,
                                op1=ALU.mult,
                            )
                    else:
                        nc.vector.tensor_tensor(hout[:], tmp[:], sg[:], ALU.mult)
                    hin = hout

                # masked mean pool + MLP head
                psG = psD.tile([1, H], dt.float32, tag="head")
                for ci in range(NCH):
                    nc.tensor.matmul(
                        psG[:],
                        maskdiv[:, ci : ci + 1],
                        hin[:, ci, :],
                        start=(ci == 0),
                        stop=(ci == NCH - 1),
                    )
                gs = smallp.tile([1, H], dt.float32, tag="gs")
                nc.scalar.copy(gs[:], psG[:])
                psGT = psD.tile([H, 1], dt.float32, tag="head")
                nc.tensor.transpose(psGT[:], gs[:], ident[0:1, 0:1])
                gT = smallp.tile([H, 1], dt.float32, tag="gT")
                nc.scalar.copy(gT[:], psGT[:])
                psH1 = psD.tile([H, 1], dt.float32, tag="head")
                nc.tensor.matmul(psH1[:], Wl1s[:], gT[:], start=True, stop=True)
                g1pre = smallp.tile([H, 1], dt.float32, tag="g1pre")
                nc.scalar.activation(
                    g1pre[:], psH1[:], AF.Identity, bias=bl1c[:, 0:1]
                )
                g1sg = smallp.tile([H, 1], dt.float32, tag="g1sg")
                nc.scalar.activation(
                    g1sg[:], psH1[:], AF.Sigmoid, bias=bl1c[:, 0:1]
                )
                g1 = smallp.tile([H, 1], dt.float32, tag="g1")
                nc.vector.tensor_tensor(g1[:], g1pre[:], g1sg[:], ALU.mult)
                psO = psD.tile([OUT, 1], dt.float32, tag="head")
                nc.tensor.matmul(psO[:], Wl2s[:], g1[:], start=True, stop=True)
                nc.scalar.activation(
                    outS[:, b : b + 1], psO[:], AF.Identity, bias=bl2c[:, 0:1]
                )

            prev = None
            for b in range(n_batches):
                st = stage_load(b)
                if prev is not None:
                    stage_compute(b - 1, prev)
                prev = st
            stage_compute(n_batches - 1, prev)

            nc.sync.dma_start(out_d.ap().rearrange("b c -> c b"), outS[:])

    nc.compile()
    return nc


_NC_CACHE = {}


def _get_nc(n_batches=BPC):
    if n_batches not in _NC_CACHE:
        _NC_CACHE[n_batches] = build_nc(n_batches)
    return _NC_CACHE[n_batches]


def make_in_maps(x, adj, mask, W0, b0, W1, b1, Wl1, bl1, Wl2, bl2):
    ws = dict(
        W0=np.ascontiguousarray(W0, np.float32),
        b0=np.ascontiguousarray(b0, np.float32),
        W1=np.ascontiguousarray(W1, np.float32),
        b1=np.ascontiguousarray(b1, np.float32),
        Wl1=np.ascontiguousarray(Wl1, np.float32),
        bl1=np.ascontiguousarray(bl1, np.float32),
        Wl2=np.ascontiguousarray(Wl2, np.float32),
        bl2=np.ascontiguousarray(bl2, np.float32),
    )
    in_maps = []
    for c in range(NCORES):
        sl = slice(c * BPC, (c + 1) * BPC)
        m = dict(
            adj=np.ascontiguousarray(adj[sl], np.float32),
            x=np.ascontiguousarray(x[sl], np.float32),
            mask=np.ascontiguousarray(mask[sl], np.int32),
        )
        m.update(ws)
        in_maps.append(m)
    return in_maps


def kernel(x, adj, mask, W0, b0, W1, b1, Wl1, bl1, Wl2, bl2, **kw):
    nc = _get_nc()
    in_maps = make_in_maps(x, adj, mask, W0, b0, W1, b1, Wl1, bl1, Wl2, bl2)
    res = run_bass_kernel_spmd(nc, in_maps, core_ids=list(range(NCORES)))
    out = np.concatenate([res.results[c]["out"] for c in range(NCORES)], axis=0)
    return out.astype(np.float32)
